# revision 10
# baseline (speedup 1.0000x reference)
"""Cost-volume concat kernel for Trainium2 (8 NeuronCores, SPMD).

Problem: left/right (B=4, C=32, H=64, W=128) f32 ->
         out (B, 2C, D=48, H, W) where
  out[b, c,    d, h, w] = left [b, c, h, w]     * (w >= d)
  out[b, C+c,  d, h, w] = right[b, c, h, w - d] * (w >= d)

Sharding: 8 cores = 4 batches x 2 disparity-halves (d0 in {0, 24}).
All cores run an IDENTICAL SPMD program; the d0 shift is absorbed
host-side by pre-shifting the left input by d0 columns and stitching the
per-core output back with a d0 column offset.

Device program (per core, DL = 24 local disparity levels, R = 8 levels
per SWDGE writeback instruction):

  The store engine is kv_writeback: each instruction writes, for R
  "batch" entries (= disparity levels), 2048 rows of <=128 floats at a
  per-level column offset ctx[level] = level, and clips each row at the
  n_ctx = 128 row boundary -- which implements the w >= d mask for free
  (masked columns keep the runtime's pre-zeroed output).  Descriptors
  are striped 16-partitions-wide, so the whole 48 MiB store costs only
  ~1030 descriptors per instruction on the DMA engines.

  Left half: the writeback's source addressing walks fixed 128-element
  blocks per batch entry, so the DVE and ACT engines first build an
  R-way replicated tile whose block b holds the left image pre-shifted
  by b columns; instruction group g then reads it at base offset g*R,
  giving level g*R+b its (g*R+b)-column shifted source.  Copies are
  split into row-halves so they start as soon as half the left image
  has landed; a dummy activation at t=0 prefetches the ACT table load.

  Right half: every level reads the *unshifted* right rows, so no
  replication is needed: with the output batch stride skewed by one
  row (level_stride + row_pitch), batch entry b's fixed source block
  j+b lands exactly on output row 16p+j+b of level g*R+b.  The skew
  leaves rows [0, b) of each level unwritten and spills reads past the
  loaded rows (garbage) into the first rows of the following level;
  one DRAM->DRAM fixup DMA (192 rows, 0.4% of the output, from a
  host-staged strip) repairs the first R rows of every level after the
  right-half writebacks finish.

Host side only shards inputs and unshards outputs (transpose the
level-major device layout back to [C, DL, H, W] order and place at the
d0 column offset; masked regions come from the zero-initialised array).
"""

import sys

for _p in ("/opt/trn_rl_repo",):
    if _p not in sys.path:
        sys.path.append(_p)

import numpy as np

import concourse.bacc as bacc
import concourse.bass as bass
import concourse.mybir as mybir
from concourse.ap import AP
from concourse.bass_utils import run_bass_kernel_spmd

B, C, H, W = 4, 32, 64, 128
D = 48
NCORES = 8
DL = D // 2            # 24 disparity levels per core
R = 6                  # levels per kv_writeback instruction
NG = DL // R           # instruction groups per image half
NROW = C * H           # 2048 (c,h) rows
RPP = NROW // 128      # 16 rows per SBUF partition
HPP = RPP // 2         # rows per copy half
PITCH = R * W          # per-row pitch in the replicated left tile
TILE2 = RPP * PITCH    # replicated tile free size (elements)
PAD2 = 32              # tail pad so shifted base offsets stay in bounds
LVL = NROW * W         # 262144 elements per level
RT_EXT = (RPP + R - 1) * W    # right-tile AP extent incl. skew margin
FIXR = R               # fixup rows per level
YR_EXT = DL * LVL + 1024      # right output incl. skew-overflow pad
DVE_COPIES = (0, 1, 2, 3)  # DVE copies are ~2x cheaper than ACT's
ACT_COPIES = (4, 5)

_F32 = mybir.dt.float32
_I32 = mybir.dt.int32

_NC_CACHE = {}


def _build_nc():
    nc = bacc.Bacc(None)

    xl = nc.dram_tensor("xl", [128, RPP * W], _F32, kind="ExternalInput")
    xrm = nc.dram_tensor("xrm", [128, RT_EXT], _F32, kind="ExternalInput")
    ci = nc.dram_tensor("ci", [128, DL], _I32, kind="ExternalInput")
    fx = nc.dram_tensor("fx", [DL, FIXR, W], _F32, kind="ExternalInput")
    yl = nc.dram_tensor("yl", [DL, C, H, W], _F32, kind="ExternalOutput")
    yr = nc.dram_tensor("yr", [YR_EXT], _F32, kind="ExternalOutput")

    lt = nc.alloc_sbuf_tensor("lt", [128, RPP * W], _F32)
    rt = nc.alloc_sbuf_tensor("rt", [128, RT_EXT], _F32)
    lt2 = nc.alloc_sbuf_tensor("lt2", [128, TILE2 + PAD2], _F32)
    cis = nc.alloc_sbuf_tensor("cis", [128, DL], _I32)
    junk = nc.alloc_sbuf_tensor("junk", [128, 1], _F32)

    def left_copy(eng, b, half):
        """Row-half of copy b of the replicated left tile, pre-shifted
        by b columns (the shifted tail is never read, no zero-fill)."""
        n = W - b
        dst = AP(
            lt2, b * W + half * HPP * PITCH,
            [[TILE2 + PAD2, 128], [PITCH, HPP], [1, n]],
        )
        src = AP(
            lt, b + half * HPP * W,
            [[RPP * W, 128], [W, HPP], [1, n]],
        )
        if eng is nc.scalar:
            return eng.copy(dst, src)
        return eng.tensor_scalar_add(dst, src, 0.0)

    def kv_left(g):
        in_ap = AP(
            lt2, g * R,
            [[TILE2 + PAD2, 128], [PITCH, RPP], [W, R], [1, W]],
        )
        out_ap = AP(
            yl, g * R * LVL,
            [[LVL, R], [RPP * W, 128], [W, RPP], [1, W]],
        )
        return nc.gpsimd.kv_writeback(out_ap, in_ap, cis[:, g * R:(g + 1) * R])

    def kv_right(g):
        # Skewed: batch entry b reads source rows j+b and writes output
        # rows 16p+j+b of level g*R+b via batch stride LVL + W.
        in_ap = AP(rt, 0, [[RT_EXT, 128], [W, RPP], [W, R], [1, W]])
        out_ap = AP(
            yr, g * R * LVL,
            [[LVL + W, R], [RPP * W, 128], [W, RPP], [1, W]],
        )
        return nc.gpsimd.kv_writeback(out_ap, in_ap, cis[:, g * R:(g + 1) * R])

    with (
        nc.Block() as block,
        nc.semaphore("ld_sem") as ld_sem,
        nc.semaphore("ryl_sem") as ryl_sem,
        nc.semaphore("kv_sem") as kv_sem,
        nc.semaphore("fx_sem") as fx_sem,
    ):
        @block.sync
        def _(sync: bass.BassEngine):
            sync.dma_start(out=cis[:], in_=ci[:]).then_inc(ld_sem, 16)
            sync.dma_start(out=rt[:], in_=xrm[:]).then_inc(ld_sem, 16)
            sync.dma_start(
                out=lt[:, :HPP * W], in_=xl[:, :HPP * W]
            ).then_inc(ld_sem, 16)
            sync.dma_start(
                out=lt[:, HPP * W:], in_=xl[:, HPP * W:]
            ).then_inc(ld_sem, 16)
            # Repair the skewed right-half edge rows once its writebacks
            # have all landed.
            sync.wait_ge(kv_sem, 16 * NG)
            sync.dma_start(
                out=AP(yr, 0, [[LVL, DL], [W, FIXR], [1, W]]), in_=fx[:]
            ).then_inc(fx_sem, 16)
            sync.wait_ge(fx_sem, 16)

        @block.vector
        def _(v: bass.BassVectorEngine):
            v.wait_ge(ld_sem, 48)
            for b in DVE_COPIES:
                left_copy(v, b, 0).then_inc(ryl_sem, 1)
            v.wait_ge(ld_sem, 64)
            for b in DVE_COPIES:
                left_copy(v, b, 1).then_inc(ryl_sem, 1)

        @block.scalar
        def _(s: bass.BassScalarEngine):
            # Dummy first activation so Bacc's table load runs at t=0
            # instead of after the lt-half wait.
            s.copy(junk[:], junk[:])
            s.wait_ge(ld_sem, 48)
            for b in ACT_COPIES:
                left_copy(s, b, 0).then_inc(ryl_sem, 1)
            s.wait_ge(ld_sem, 64)
            for b in ACT_COPIES:
                left_copy(s, b, 1).then_inc(ryl_sem, 1)

        @block.gpsimd
        def _(g: bass.BassGpSimd):
            g.wait_ge(ld_sem, 32)
            for grp in range(NG):
                kv_right(grp).then_inc(kv_sem, 16)
            g.wait_ge(ryl_sem, 2 * R)
            for grp in range(NG):
                kv_left(grp).then_inc(kv_sem, 16)
            g.wait_ge(kv_sem, 16 * 2 * NG)

    nc.finalize()
    return nc


def _get_nc():
    if "nc" not in _NC_CACHE:
        _NC_CACHE["nc"] = _build_nc()
    return _NC_CACHE["nc"]


def _run(left, right, **spmd_kwargs):
    left = np.ascontiguousarray(np.asarray(left), dtype=np.float32)
    right = np.ascontiguousarray(np.asarray(right), dtype=np.float32)

    ci = np.tile(np.arange(DL, dtype=np.int32), (128, 1))
    in_maps = []
    for k in range(NCORES):
        b, q = divmod(k, 2)
        d0 = DL * q
        xl = np.zeros((C, H, W), np.float32)
        xl[:, :, :W - d0] = left[b, :, :, d0:]
        # Fixup strip: correctly masked/shifted first FIXR rows of every
        # level of the right half.
        fxa = np.zeros((DL, FIXR, W), np.float32)
        for lv in range(DL):
            fxa[lv, :, lv:] = right[b, 0, 0:FIXR, 0:W - lv]
        rflat = np.zeros(NROW * W + (R - 1) * W, np.float32)
        rflat[:NROW * W] = right[b].reshape(-1)
        xrm = np.stack(
            [rflat[p * RPP * W: p * RPP * W + RT_EXT] for p in range(128)]
        )
        in_maps.append(
            {
                "xl": xl.reshape(128, RPP * W),
                "xrm": xrm,
                "ci": ci,
                "fx": fxa,
            }
        )

    res = run_bass_kernel_spmd(
        _get_nc(), in_maps, core_ids=list(range(NCORES)), **spmd_kwargs
    )

    out = np.zeros((B, 2 * C, D, H, W), np.float32)
    for k in range(NCORES):
        b, q = divmod(k, 2)
        d0 = DL * q
        ylr = res.results[k]["yl"].transpose(1, 0, 2, 3)
        yrr = (
            res.results[k]["yr"][:DL * LVL]
            .reshape(DL, C, H, W)
            .transpose(1, 0, 2, 3)
        )
        out[b, 0:C, d0:d0 + DL, :, d0:] = ylr[:, :, :, :W - d0]
        out[b, C:, d0:d0 + DL, :, d0:] = yrr[:, :, :, :W - d0]
    return out, res


def kernel(left, right):
    out, _ = _run(left, right)
    return out


# revision 23
# speedup vs baseline: 1.0163x; 1.0163x over previous
"""Cost-volume concat kernel for Trainium2 (8 NeuronCores, SPMD).

Problem: left/right (B=4, C=32, H=64, W=128) f32 ->
         out (B, 2C, D=48, H, W) where
  out[b, c,    d, h, w] = left [b, c, h, w]     * (w >= d)
  out[b, C+c,  d, h, w] = right[b, c, h, w - d] * (w >= d)

Sharding: 8 cores = 4 batches x 2 disparity-halves (d0 in {0, 24}).
All cores run an IDENTICAL SPMD program; the d0 shift is absorbed
host-side by pre-shifting the left input by d0 columns and stitching the
per-core output back with a d0 column offset.

Device program (per core, DL = 24 local disparity levels, R = 8 levels
per SWDGE writeback instruction):

  The store engine is kv_writeback: each instruction writes, for R
  "batch" entries (= disparity levels), 2048 rows of <=128 floats at a
  per-level column offset ctx[level] = level, and clips each row at the
  n_ctx = 128 row boundary -- which implements the w >= d mask for free
  (masked columns keep the runtime's pre-zeroed output).  Descriptors
  are striped 16-partitions-wide, so the whole 48 MiB store costs only
  ~1030 descriptors per instruction on the DMA engines.

  Left half: the writeback's source addressing walks fixed 128-element
  blocks per batch entry, so the DVE and ACT engines first build an
  R-way replicated tile whose block b holds the left image pre-shifted
  by b columns; instruction group g then reads it at base offset g*R,
  giving level g*R+b its (g*R+b)-column shifted source.  Copies are
  split into row-halves so they start as soon as half the left image
  has landed; a dummy activation at t=0 prefetches the ACT table load.

  Right half: every level reads the *unshifted* right rows, so no
  replication is needed: with the output batch stride skewed by one
  row (level_stride + row_pitch), batch entry b's fixed source block
  j+b lands exactly on output row 16p+j+b of level g*R+b.  The skew
  leaves rows [0, b) of each level unwritten and spills reads past the
  loaded rows (garbage) into the first rows of the following level;
  one DRAM->DRAM fixup DMA (192 rows, 0.4% of the output, from a
  host-staged strip) repairs the first R rows of every level after the
  right-half writebacks finish.

Host side only shards inputs and unshards outputs (transpose the
level-major device layout back to [C, DL, H, W] order and place at the
d0 column offset; masked regions come from the zero-initialised array).
"""

import sys

for _p in ("/opt/trn_rl_repo",):
    if _p not in sys.path:
        sys.path.append(_p)

import numpy as np

import concourse.bacc as bacc
import concourse.bass as bass
import concourse.mybir as mybir
from concourse.ap import AP
from concourse.bass_utils import run_bass_kernel_spmd

B, C, H, W = 4, 32, 64, 128
D = 48
NCORES = 8
DL = D // 2            # 24 disparity levels per core
R = 6                  # levels per kv_writeback instruction
NG = DL // R           # instruction groups per image half
NROW = C * H           # 2048 (c,h) rows
RPP = NROW // 128      # 16 rows per SBUF partition
QPP = RPP // 4         # rows per copy quarter
PITCH = R * W          # per-row pitch in the replicated left tile
TILE2 = RPP * PITCH    # replicated tile free size (elements)
PAD2 = 32              # tail pad so shifted base offsets stay in bounds
LVL = NROW * W         # 262144 elements per level
RT_EXT = (RPP + R - 1) * W    # right-tile AP extent incl. skew margin
FIXR = R               # fixup rows per level
YR_EXT = DL * LVL + 1024      # right output incl. skew-overflow pad
DVE_COPIES = (1, 2, 3)  # DVE copies are ~2x cheaper than ACT's
ACT_COPIES = (4, 5)

# Dev-time ablation knobs (all False for the real kernel).
ABL_NO_FIXUP = False
ABL_NO_COPIES = False
ABL_NO_YR = False
ABL_NO_YL = False
ABL_TINY_LOADS = False
LOAD_K = 1             # lt quarters loaded before the rt image

_F32 = mybir.dt.float32
_I32 = mybir.dt.int32

def _q_thresholds():
    # ld_sem value at which lt quarter q is resident, given LOAD_K
    thr = []
    for q in range(4):
        pos = 1 + q + (1 if q >= LOAD_K else 0)  # cis + preceding DMAs
        thr.append(16 * (pos + 1))
    return tuple(thr)


_NC_CACHE = {}


def _build_nc():
    nc = bacc.Bacc(None)

    xl = nc.dram_tensor("xl", [128, RPP * W], _F32, kind="ExternalInput")
    xrm = nc.dram_tensor("xrm", [128, RT_EXT], _F32, kind="ExternalInput")
    ci = nc.dram_tensor("ci", [128, DL], _I32, kind="ExternalInput")
    fx = nc.dram_tensor("fx", [DL, FIXR, W], _F32, kind="ExternalInput")
    yl = nc.dram_tensor("yl", [DL, C, H, W], _F32, kind="ExternalOutput")
    yr = nc.dram_tensor("yr", [YR_EXT], _F32, kind="ExternalOutput")

    rt = nc.alloc_sbuf_tensor("rt", [128, RT_EXT], _F32)
    lt2 = nc.alloc_sbuf_tensor("lt2", [128, TILE2 + PAD2], _F32)
    cis = nc.alloc_sbuf_tensor("cis", [128, DL], _I32)
    junk = nc.alloc_sbuf_tensor("junk", [128, 1], _F32)

    def left_copy(eng, b, q):
        """Row-quarter of copy b of the replicated left tile, pre-shifted
        by b columns, sourced from block 0 (the loaded image); the
        shifted tail is never read, so no zero-fill."""
        n = W - b
        dst = AP(
            lt2, b * W + q * QPP * PITCH,
            [[TILE2 + PAD2, 128], [PITCH, QPP], [1, n]],
        )
        src = AP(
            lt2, b + q * QPP * PITCH,
            [[TILE2 + PAD2, 128], [PITCH, QPP], [1, n]],
        )
        if eng is nc.scalar:
            return eng.copy(dst, src)
        return eng.tensor_scalar_add(dst, src, 0.0)

    def kv_left(g):
        in_ap = AP(
            lt2, g * R,
            [[TILE2 + PAD2, 128], [PITCH, RPP], [W, R], [1, W]],
        )
        out_ap = AP(
            yl, g * R * LVL,
            [[LVL, R], [RPP * W, 128], [W, RPP], [1, W]],
        )
        return nc.gpsimd.kv_writeback(out_ap, in_ap, cis[:, g * R:(g + 1) * R])

    def kv_right(g, **kw):
        # Skewed: batch entry b reads source rows j+b and writes output
        # rows 16p+j+b of level g*R+b via batch stride LVL + W.
        in_ap = AP(rt, 0, [[RT_EXT, 128], [W, RPP], [W, R], [1, W]])
        out_ap = AP(
            yr, g * R * LVL,
            [[LVL + W, R], [RPP * W, 128], [W, RPP], [1, W]],
        )
        return nc.gpsimd.kv_writeback(
            out_ap, in_ap, cis[:, g * R:(g + 1) * R], **kw
        )

    with (
        nc.Block() as block,
        nc.semaphore("ld_cis") as ld_cis,
        nc.semaphore("ld_rt") as ld_rt,
        nc.semaphore("ld_q0") as ld_q0,
        nc.semaphore("ld_q1") as ld_q1,
        nc.semaphore("ld_q2") as ld_q2,
        nc.semaphore("ld_q3") as ld_q3,
        nc.semaphore("ryl_sem") as ryl_sem,
        nc.semaphore("kv_sem") as kv_sem,
        nc.semaphore("fx_sem") as fx_sem,
        nc.semaphore("prep_sem") as prep_sem,
    ):
        @block.sync
        def _(sync: bass.BassEngine):
            ld_q = (ld_q0, ld_q1, ld_q2, ld_q3)
            if ABL_TINY_LOADS:
                sync.dma_start(out=cis[:, :2], in_=ci[:, :2]).then_inc(ld_cis, 16)
                sync.dma_start(out=rt[:, :2], in_=xrm[:, :2]).then_inc(ld_rt, 16)
                for q in range(4):
                    sync.dma_start(
                        out=lt2[:, 2 * q + 2:2 * q + 4],
                        in_=xl[:, 2 * q + 2:2 * q + 4],
                    ).then_inc(ld_q[q], 16)
            else:
                def ltq(q):
                    # Load straight into block 0 of the replicated tile.
                    return sync.dma_start(
                        out=AP(
                            lt2, q * QPP * PITCH,
                            [[TILE2 + PAD2, 128], [PITCH, QPP], [1, W]],
                        ),
                        in_=xl[:, q * QPP * W:(q + 1) * QPP * W],
                    )

                sync.dma_start(out=cis[:], in_=ci[:]).then_inc(ld_cis, 16)
                for q in range(LOAD_K):
                    ltq(q).then_inc(ld_q[q], 16)
                sync.dma_start(out=rt[:], in_=xrm[:]).then_inc(ld_rt, 16)
                for q in range(LOAD_K, 4):
                    ltq(q).then_inc(ld_q[q], 16)
            # Repair the skewed right-half edge rows once its writebacks
            # have all landed.
            if not ABL_NO_FIXUP:
                sync.wait_ge(kv_sem, 16 * NG * (0 if ABL_NO_YR else 1))
                sync.dma_start(
                    out=AP(yr, 0, [[LVL, DL], [W, FIXR], [1, W]]), in_=fx[:]
                ).then_inc(fx_sem, 16)
                sync.wait_ge(fx_sem, 16)

        @block.vector
        def _(v: bass.BassVectorEngine):
            ld_q = (ld_q0, ld_q1, ld_q2, ld_q3)
            if ABL_NO_COPIES:
                v.wait_ge(ld_q[3], 16)
                for _ in range(4 * len(DVE_COPIES)):
                    v.sem_inc(ryl_sem, 1)
                return
            for q in range(4):
                v.wait_ge(ld_q[q], 16)
                for b in DVE_COPIES:
                    left_copy(v, b, q).then_inc(ryl_sem, 1)

        @block.scalar
        def _(s: bass.BassScalarEngine):
            ld_q = (ld_q0, ld_q1, ld_q2, ld_q3)
            if ABL_NO_COPIES:
                s.wait_ge(ld_q[3], 16)
                for _ in range(4 * len(ACT_COPIES)):
                    s.sem_inc(ryl_sem, 1)
                return
            # Dummy first activation so Bacc's table load runs at t=0
            # instead of after the lt-quarter wait.
            s.copy(junk[:], junk[:])
            for q in range(4):
                s.wait_ge(ld_q[q], 16)
                for b in ACT_COPIES:
                    left_copy(s, b, q).then_inc(ryl_sem, 1)

        @block.gpsimd
        def _(g: bass.BassGpSimd):
            nkv = 0
            g.wait_ge(ld_cis, 16)
            g.wait_ge(ld_rt, 16)
            if not ABL_NO_YR:
                for grp in range(NG):
                    kv_right(grp).then_inc(kv_sem, 16)
                nkv += NG
            g.wait_ge(ryl_sem, 4 * (R - 1))
            if not ABL_NO_YL:
                for grp in range(NG):
                    kv_left(grp).then_inc(kv_sem, 16)
                nkv += NG
            g.wait_ge(kv_sem, 16 * nkv)

    nc.finalize()
    return nc


def _get_nc():
    if "nc" not in _NC_CACHE:
        _NC_CACHE["nc"] = _build_nc()
    return _NC_CACHE["nc"]


def _run(left, right, **spmd_kwargs):
    left = np.ascontiguousarray(np.asarray(left), dtype=np.float32)
    right = np.ascontiguousarray(np.asarray(right), dtype=np.float32)

    ci = np.tile(np.arange(DL, dtype=np.int32), (128, 1))
    in_maps = []
    for k in range(NCORES):
        b, q = divmod(k, 2)
        d0 = DL * q
        xl = np.zeros((C, H, W), np.float32)
        xl[:, :, :W - d0] = left[b, :, :, d0:]
        # Fixup strip: correctly masked/shifted first FIXR rows of every
        # level of the right half.
        fxa = np.zeros((DL, FIXR, W), np.float32)
        for lv in range(DL):
            fxa[lv, :, lv:] = right[b, 0, 0:FIXR, 0:W - lv]
        rflat = np.zeros(NROW * W + (R - 1) * W, np.float32)
        rflat[:NROW * W] = right[b].reshape(-1)
        xrm = np.stack(
            [rflat[p * RPP * W: p * RPP * W + RT_EXT] for p in range(128)]
        )
        in_maps.append(
            {
                "xl": xl.reshape(128, RPP * W),
                "xrm": xrm,
                "ci": ci,
                "fx": fxa,
            }
        )

    res = run_bass_kernel_spmd(
        _get_nc(), in_maps, core_ids=list(range(NCORES)), **spmd_kwargs
    )

    out = np.zeros((B, 2 * C, D, H, W), np.float32)
    for k in range(NCORES):
        b, q = divmod(k, 2)
        d0 = DL * q
        ylr = res.results[k]["yl"].transpose(1, 0, 2, 3)
        yrr = (
            res.results[k]["yr"][:DL * LVL]
            .reshape(DL, C, H, W)
            .transpose(1, 0, 2, 3)
        )
        out[b, 0:C, d0:d0 + DL, :, d0:] = ylr[:, :, :, :W - d0]
        out[b, C:, d0:d0 + DL, :, d0:] = yrr[:, :, :, :W - d0]
    return out, res


def kernel(left, right):
    out, _ = _run(left, right)
    return out
